# revision 19
# baseline (speedup 1.0000x reference)
"""MiniCausalAttention on 8 NeuronCores (Trainium2, Bass/Tile).

Problem: x[4,2048,1024] fp32; q/k/v = x@w+b; causal softmax(q k^T/sqrt(D)) @ v.

Sharding: 8 cores = (batch b in 0..3) x (half h in 0..1). Core (b,h) handles
query tiles g = 2t+h for local t in 0..7 (interleaved 128-row tiles), so every
core sees the same causal extents -> one SPMD program, perfectly balanced.

Transpose-free formulation. With M = Wq Wk^T and u = Wk bq (host-precomputed),
the softmax-effective scores are S = x_q M x^T + 1 (x) (x u)^T (per-query
terms cancel). Computing the TRANSPOSED scores instead,
  S^T = x (M^T x_q^T + u 1^T) = x H,      H = G^T + u 1^T,  G^T = M^T x_q^T,
makes every downstream operand come out in exactly the layout the next matmul
needs, so the kernel contains zero PE transposes:
  phase A   G^T[d,q]   = lhsT(m) @ x_q^T        -> +u -> H in SBUF (bf16)
  S-step    S^T[k,q]   = lhsT(x^T) @ H          -> mask+exp -> P^T (bf16)
  Z-step    Z^T[d',q]  = lhsT(x rows) @ P^T      (attention @ x, un-projected)
  O-step    O[q,d]     = lhsT(Z^T) @ Wv          (query-major, DMA-ready)
Row-sums ride the Z-step as a 1-column ones matmul; normalization (/rowsum,
+bv) runs on the host after gather (O(L*D) work, like the unshard reshape).

q-tiles are processed in pairs (256-col moving operands) over key tiles
kt < 4p+4 with per-core mask data resolving h and the diagonal. PSUM (8
banks): S x3, Z-pass x2, O/Z-pass2 x2 (shared tags, phase-shifted), rowsum x1.
Z^T runs in two d-half passes to stay in 2 banks; pass 2 shares tags with the
O matmuls of the previous pair, which interleave one pair later.
"""

import sys

if "/opt/trn_rl_repo" not in sys.path:
    sys.path.insert(0, "/opt/trn_rl_repo")

import numpy as np
import ml_dtypes

import concourse.bass as bass  # noqa: F401
import concourse.tile as tile
from concourse import bacc, mybir
from concourse.bass_utils import run_bass_kernel_spmd

BF16 = mybir.dt.bfloat16
F32 = mybir.dt.float32
AF = mybir.ActivationFunctionType

B, L, D = 4, 2048, 1024
P = 128
SCALE = 1.0 / 32.0   # 1/sqrt(D)
NEG = -30000.0       # exp(SCALE*NEG) == 0 in f32
NSPIN = 18           # PE warmup spins covering the input-DMA window

_CACHED = {}


def build_nc():
    nc = bacc.Bacc(None, target_bir_lowering=False)

    xt = nc.declare_dram_parameter("xt", [2, P, 8 * 1024], BF16, isOutput=False)
    xtq = nc.declare_dram_parameter("xtq", [4, P, 2 * 1024], BF16, isOutput=False)
    m2 = nc.declare_dram_parameter("m2", [8, P, 1024], BF16, isOutput=False)
    xr = nc.declare_dram_parameter("xr", [4, P, 4 * 1024], BF16, isOutput=False)
    wvd = nc.declare_dram_parameter("wv", [P, 8 * 1024], BF16, isOutput=False)
    ud = nc.declare_dram_parameter("u", [P, 8], F32, isOutput=False)
    maskd = nc.declare_dram_parameter("mask", [P, 4 * 256], F32, isOutput=False)
    outd = nc.declare_dram_parameter("out", [8 * P, D], BF16, isOutput=True)
    rsd = nc.declare_dram_parameter("rs", [1, 8 * P], F32, isOutput=True)

    with tile.TileContext(nc) as tc:
        with tc.tile_pool(name="persist", bufs=1) as persist:
            # DMA-contiguous layouts: each input DMA writes one fully
            # contiguous free-range per partition (8-16KB packets; strided
            # packets starve in the DMA engines' packet round-robin).
            xt_sb = persist.tile([P, 2, 8, 1024], BF16)  # x^T: [d, half, ct, tok]
            xtq_sb = persist.tile([P, 4, 8, 256], BF16)  # x_q^T: [d, qq, ct, q]
            m_sb = persist.tile([P, 8, 8, P], BF16)     # M: [d'-in-chunk, dt, ct, dcol]
            xr_sb = persist.tile([P, 16, 1024], BF16)   # x rows: [tok-in-tile, kt, d]
            wv_sb = persist.tile([P, 8, 1024], BF16)    # Wv: [d'-in-chunk, s, d]
            u_sb = persist.tile([P, 8], F32)            # Wk bq: [d-in-chunk, dt]
            mask_sb = persist.tile([P, 4, 256], F32)    # diag masks: [k, r, qcol]
            h_sb = persist.tile([P, 8, 1024], BF16)     # H = G^T + u: [d, dt, qcol]
            p_sb = persist.tile([P, 16, 256], BF16)     # P^T per pair: [k, kt, qcol]
            zt_sb = persist.tile([P, 8, 256], BF16)     # Z^T: [d'-in-chunk, s, qcol]
            rs_sb = persist.tile([1, 8 * P], F32)       # rowsums, local-q linear
            ones_sb = persist.tile([P, 1], BF16)
            warm_w = persist.tile([P, P], BF16)
            warm_x = persist.tile([P, 256], BF16)

            # Three parallel DMA streams. The two tensors gating phase A (m2,
            # xtq) lead the two queues that flow earliest (sync HWDGE and
            # gpsimd SWDGE); the scalar HWDGE queue gets arbitration-starved
            # behind them, so it carries only tensors needed later (masks at
            # first S-block, wv at first O).
            nc.sync.dma_start(out=m_sb[:, 0:1, :, :], in_=m2[0, :, :])
            nc.sync.dma_start(out=xtq_sb[:, 0], in_=xtq[0, :, :])
            for i in range(1, 8):
                nc.sync.dma_start(out=m_sb[:, i:i + 1, :, :], in_=m2[i, :, :])
            nc.sync.dma_start(out=xtq_sb[:, 1], in_=xtq[1, :, :])
            nc.sync.dma_start(out=xtq_sb[:, 2], in_=xtq[2, :, :])
            nc.sync.dma_start(out=xtq_sb[:, 3], in_=xtq[3, :, :])
            nc.sync.dma_start(out=xt_sb[:, 0], in_=xt[0, :, :])
            nc.sync.dma_start(out=xt_sb[:, 1], in_=xt[1, :, :])
            nc.gpsimd.dma_start(out=u_sb, in_=ud[:, :])
            nc.gpsimd.dma_start(out=mask_sb, in_=maskd[:, :])


            nc.vector.memset(ones_sb, 1.0)
            nc.vector.memset(warm_w, 1.0)
            nc.vector.memset(warm_x, 0.125)

            with tc.tile_pool(name="bwork", bufs=2) as bwork, \
                 tc.tile_pool(name="ps", bufs=1, space="PSUM") as ps:

                # HAM warmup + DMA-window filler: keeps the PE busy (and at
                # 2.4 GHz) until the first real operands land.
                for i in range(NSPIN):
                    junk = ps.tile([P, 256], F32, tag="s", bufs=3, name="junk")
                    nc.tensor.matmul(junk, warm_w, warm_x, start=True, stop=True)

                # ---------------- Phase A: H = M^T x_q^T + u ----------------
                ztags = ["za", "zb", "oa", "ob"]
                for qq in range(4):
                    for dt in range(8):
                        pg = ps.tile([P, 256], F32, tag=ztags[(qq * 8 + dt) % 4],
                                     name="pg")
                        for ct in range(8):
                            nc.tensor.matmul(
                                pg,
                                m_sb[:, dt, ct, :],
                                xtq_sb[:, qq, ct, :],
                                start=(ct == 0),
                                stop=(ct == 7),
                            )
                        nc.vector.tensor_scalar_add(
                            h_sb[:, dt, qq * 256:(qq + 1) * 256], pg,
                            u_sb[:, dt:dt + 1])
                        if qq == 0 and dt == 6:
                            # xr/wv are not needed until ~40us, but their
                            # large packets win the DMA engines' packet
                            # round-robin and starve the phase-A gating
                            # transfers. Hold them until G's second quarter:
                            # the touches below READ the h chunk written
                            # above (a real RAW dep -- the tile scheduler
                            # reorders anything without one), and the DMAs
                            # wait on the touches via WAW overlap.
                            for q4 in range(4):
                                nc.vector.tensor_copy(
                                    xr_sb[:, 4 * q4, 0:1],
                                    h_sb[:, dt, qq * 256:qq * 256 + 1])
                            nc.vector.tensor_copy(
                                wv_sb[:, 0, 0:1],
                                h_sb[:, dt, qq * 256:qq * 256 + 1])
                            for q4 in range(4):
                                nc.scalar.dma_start(
                                    out=xr_sb[:, 4 * q4:4 * q4 + 4, :],
                                    in_=xr[q4, :, :])
                            nc.scalar.dma_start(out=wv_sb, in_=wvd[:, :])

                # ------------- Phase B: attention per q-tile pair -----------
                # PSUM start=True zeroes a whole 2KB bank region, so every
                # accumulating tile gets its own bank: Z^T runs as 4 passes of
                # 2 slices on tags za..ob (1 bank each); the O matmuls reuse
                # za/zb right after pass 3's copies; the next pair's pass 0
                # reuses them again after the o copies. All sequential.
                for p in range(4):
                    K = 4 * p + 4
                    rs_ps = ps.tile([1, 256], F32, tag="rs", name="rs_ps")

                    def s_block(kt):
                        s_ps = ps.tile([P, 256], F32, tag="s", bufs=3, name="s_ps")
                        for ct in range(8):
                            nc.tensor.matmul(
                                s_ps,
                                xt_sb[:, kt // 8, ct, (kt % 8) * P:(kt % 8 + 1) * P],
                                h_sb[:, ct, p * 256:(p + 1) * 256],
                                start=(ct == 0),
                                stop=(ct == 7),
                            )
                        if kt >= 4 * p:
                            nc.vector.tensor_add(s_ps, s_ps,
                                                 mask_sb[:, kt - 4 * p, :])
                        nc.scalar.activation(p_sb[:, kt, :], s_ps, AF.Exp,
                                             scale=SCALE)

                    def z_pass(j, zp):
                        # slices 2j, 2j+1 of Z^T; rowsum rides pass 0
                        for kt in range(K):
                            for sl in range(2):
                                s = 2 * j + sl
                                nc.tensor.matmul(
                                    zp[sl],
                                    xr_sb[:, kt, s * P:(s + 1) * P],
                                    p_sb[:, kt, :],
                                    start=(kt == 0),
                                    stop=(kt == K - 1),
                                )
                            if j == 0:
                                nc.tensor.matmul(rs_ps, ones_sb, p_sb[:, kt, :],
                                                 start=(kt == 0),
                                                 stop=(kt == K - 1))
                        for sl in range(2):
                            nc.vector.tensor_copy(zt_sb[:, 2 * j + sl, :], zp[sl])

                    # All S blocks first (exp chases one block behind on the
                    # scalar engine), then the Z passes stream P^T. Pass tags
                    # alternate za/zb <-> oa/ob so pass j+1 never waits on
                    # pass j's PSUM->SBUF copies.
                    for kt in range(K):
                        s_block(kt)
                    for j in range(4):
                        z_pass(j, [ps.tile([P, 256], F32,
                                           tag=ztags[2 * (j % 2) + sl],
                                           name="zp") for sl in range(2)])
                    nc.scalar.copy(rs_sb[:, p * 256:(p + 1) * 256], rs_ps)
                    if p == 3:
                        nc.sync.dma_start(out=rsd[:, :], in_=rs_sb)

                    # O for pair p: zt copies chase the Z passes, so the s=0..5
                    # reads are ready and 6..7 land under the first O matmuls.
                    for qh in range(2):
                        o_sb = bwork.tile([P, D], BF16, tag="o")
                        for dc in range(2):
                            po = ps.tile([P, 512], F32, tag=ztags[dc], name="po")
                            for s in range(8):
                                nc.tensor.matmul(
                                    po,
                                    zt_sb[:, s, qh * P:(qh + 1) * P],
                                    wv_sb[:, s, dc * 512:(dc + 1) * 512],
                                    start=(s == 0),
                                    stop=(s == 7),
                                )
                            nc.scalar.copy(o_sb[:, dc * 512:(dc + 1) * 512], po)
                            t = 2 * p + qh
                            nc.sync.dma_start(
                                out=outd[t * P:(t + 1) * P,
                                         dc * 512:(dc + 1) * 512],
                                in_=o_sb[:, dc * 512:(dc + 1) * 512])


    nc.finalize()
    return nc


def _prep_inputs(x, wq, bq, wk, bk, wv, bv):
    bf = ml_dtypes.bfloat16
    wq32 = np.asarray(wq, np.float32)
    wk32 = np.asarray(wk, np.float32)
    m_host = (wq32 @ wk32.T).astype(bf)                 # Wq Wk^T  [D, D]
    u_host = (wk32 @ np.asarray(bq, np.float32))        # Wk bq    [D]

    # M in dt-major chunks: m2[dh][p, dtm*1024 + ct*128 + c]
    #   = M[ct*128+p, (dh*4+dtm)*128 + c]
    m2 = np.ascontiguousarray(
        m_host.reshape(8, P, 8, P).transpose(2, 1, 0, 3).reshape(8, P, 1024))

    wv_b = (np.asarray(wv, np.float32).astype(bf)
            .reshape(8, P, 1024).transpose(1, 0, 2).reshape(P, 8192))
    wv_b = np.ascontiguousarray(wv_b)
    u2 = np.ascontiguousarray(u_host.reshape(8, P).T).astype(np.float32)

    # Diagonal-region masks per h: keep iff r*128+ki <= (h+2*jh)*128 + j
    ki = np.arange(P)[:, None, None]
    r = np.arange(4)[None, :, None]
    j = np.arange(256)[None, None, :]
    masks = [np.where(r * P + ki <= (h + 2 * (j // P)) * P + (j % P),
                      0.0, NEG).astype(np.float32).reshape(P, 4 * 256)
             for h in range(2)]

    in_maps = []
    for core in range(8):
        b, h = core // 2, core % 2
        xb = np.asarray(x[b], np.float32)
        xbT = xb.T.astype(bf)                            # [D, L]
        # x^T token-halves: xt[half][p, ct*1024+tok]
        xt_d = np.ascontiguousarray(
            xbT.reshape(8, P, 2, 1024).transpose(2, 1, 0, 3).reshape(2, P, 8192))
        # q-columns of x^T (local tile t -> global 2t+h), qc-halves
        qcols = ((np.arange(8)[:, None] * 2 + h) * P
                 + np.arange(P)[None, :]).ravel()
        xqT = np.ascontiguousarray(xbT[:, qcols])        # [D, 1024]
        xtq_d = np.ascontiguousarray(
            xqT.reshape(8, P, 4, 256).transpose(2, 1, 0, 3).reshape(4, P, 2048))
        # x rows in kt-quarters: xr[q4][p, kti*1024+d]
        xr_d = np.ascontiguousarray(
            xb.astype(bf).reshape(4, 4, P, 1024).transpose(0, 2, 1, 3)
            .reshape(4, P, 4096))
        in_maps.append({
            "xt": xt_d, "xtq": xtq_d, "m2": m2, "xr": xr_d, "wv": wv_b,
            "u": u2, "mask": masks[h],
        })
    return in_maps


def kernel(x, wq, bq, wk, bk, wv, bv, _trace=False, _trace_kwargs=None):
    if "nc" not in _CACHED:
        _CACHED["nc"] = build_nc()
    nc = _CACHED["nc"]
    in_maps = _prep_inputs(x, wq, bq, wk, bk, wv, bv)
    kw = {}
    if _trace:
        kw = dict(trace=True, **(_trace_kwargs or {}))
    res = run_bass_kernel_spmd(nc, in_maps, list(range(8)), **kw)
    bv32 = np.asarray(bv, np.float32)
    out = np.empty((B, L, D), np.float32)
    for core in range(8):
        b, h = core // 2, core % 2
        o = np.asarray(res.results[core]["out"]).astype(np.float32)
        rs = np.asarray(res.results[core]["rs"], np.float32).reshape(8 * P, 1)
        o = o / rs + bv32[None, :]
        out[b].reshape(16, P, D)[h::2] = o.reshape(8, P, D)
    if _trace:
        _CACHED["last_results"] = res
    return out


# revision 20
# speedup vs baseline: 1.0211x; 1.0211x over previous
"""MiniCausalAttention on 8 NeuronCores (Trainium2, Bass/Tile).

Problem: x[4,2048,1024] fp32; q/k/v = x@w+b; causal softmax(q k^T/sqrt(D)) @ v.

Sharding: 8 cores = (batch b in 0..3) x (half h in 0..1). Core (b,h) handles
query tiles g = 2t+h for local t in 0..7 (interleaved 128-row tiles), so every
core sees the same causal extents -> one SPMD program, perfectly balanced.

Transpose-free formulation. With M = Wq Wk^T and u = Wk bq (host-precomputed),
the softmax-effective scores are S = x_q M x^T + 1 (x) (x u)^T (per-query
terms cancel). Computing the TRANSPOSED scores instead,
  S^T = x (M^T x_q^T + u 1^T) = x H,      H = G^T + u 1^T,  G^T = M^T x_q^T,
makes every downstream operand come out in exactly the layout the next matmul
needs, so the kernel contains zero PE transposes:
  phase A   G^T[d,q]   = lhsT(m) @ x_q^T        -> +u -> H in SBUF (bf16)
  S-step    S^T[k,q]   = lhsT(x^T) @ H          -> mask+exp -> P^T (bf16)
  Z-step    Z^T[d',q]  = lhsT(x rows) @ P^T      (attention @ x, un-projected)
  O-step    O[q,d]     = lhsT(Z^T) @ Wv          (query-major, DMA-ready)
Row-sums ride the Z-step as a 1-column ones matmul; normalization (/rowsum,
+bv) runs on the host after gather (O(L*D) work, like the unshard reshape).

q-tiles are processed in pairs (256-col moving operands) over key tiles
kt < 4p+4 with per-core mask data resolving h and the diagonal. PSUM start=True
zeroes the whole 2KB bank, so every accumulating tile owns a bank (8 banks:
S x3, Z-pass tags x4, rowsum x1); Z^T runs as 4 passes of 2 slices, and the
O matmuls reuse two Z tags right after the pass-3 copies. Input DMAs are laid
out fully contiguous per partition (big packets win the DMA engines' packet
round-robin) with the not-yet-needed xr/wv transfers data-gated behind phase
A's second quarter so the phase-A gating transfers get the early bandwidth.
"""

import sys

if "/opt/trn_rl_repo" not in sys.path:
    sys.path.insert(0, "/opt/trn_rl_repo")

import numpy as np
import ml_dtypes

import concourse.bass as bass  # noqa: F401
import concourse.tile as tile
from concourse import bacc, mybir
from concourse.bass_utils import run_bass_kernel_spmd

BF16 = mybir.dt.bfloat16
F32 = mybir.dt.float32
AF = mybir.ActivationFunctionType

B, L, D = 4, 2048, 1024
P = 128
SCALE = 1.0 / 32.0   # 1/sqrt(D)
NEG = -30000.0       # exp(SCALE*NEG) == 0 in f32
NSPIN = 22           # PE warmup spins covering the input-DMA window

_CACHED = {}


def build_nc():
    nc = bacc.Bacc(None, target_bir_lowering=False)

    xt = nc.declare_dram_parameter("xt", [2, P, 8 * 1024], BF16, isOutput=False)
    xtq = nc.declare_dram_parameter("xtq", [4, P, 2 * 1024], BF16, isOutput=False)
    m2 = nc.declare_dram_parameter("m2", [4, P, 2 * 1024], BF16, isOutput=False)
    xr = nc.declare_dram_parameter("xr", [4, P, 4 * 1024], BF16, isOutput=False)
    wvd = nc.declare_dram_parameter("wv", [P, 8 * 1024], BF16, isOutput=False)
    ud = nc.declare_dram_parameter("u", [P, 8], F32, isOutput=False)
    maskd = nc.declare_dram_parameter("mask", [P, 4 * 256], F32, isOutput=False)
    outd = nc.declare_dram_parameter("out", [8 * P, D], BF16, isOutput=True)
    rsd = nc.declare_dram_parameter("rs", [1, 8 * P], F32, isOutput=True)

    with tile.TileContext(nc) as tc:
        with tc.tile_pool(name="persist", bufs=1) as persist:
            # DMA-contiguous layouts: each input DMA writes one fully
            # contiguous free-range per partition (8-16KB packets; strided
            # packets starve in the DMA engines' packet round-robin).
            xt_sb = persist.tile([P, 2, 8, 1024], BF16)  # x^T: [d, half, ct, tok]
            xtq_sb = persist.tile([P, 4, 8, 256], BF16)  # x_q^T: [d, qq, ct, q]
            m_sb = persist.tile([P, 8, 8, P], BF16)     # M: [d'-in-chunk, dt, ct, dcol]
            xr_sb = persist.tile([P, 16, 1024], BF16)   # x rows: [tok-in-tile, kt, d]
            wv_sb = persist.tile([P, 8, 1024], BF16)    # Wv: [d'-in-chunk, s, d]
            u_sb = persist.tile([P, 8], F32)            # Wk bq: [d-in-chunk, dt]
            mask_sb = persist.tile([P, 4, 256], F32)    # diag masks: [k, r, qcol]
            h_sb = persist.tile([P, 8, 1024], BF16)     # H = G^T + u: [d, dt, qcol]
            p_sb = persist.tile([P, 16, 256], BF16)     # P^T per pair: [k, kt, qcol]
            zt_sb = persist.tile([P, 8, 256], BF16)     # Z^T: [d'-in-chunk, s, qcol]
            rs_sb = persist.tile([1, 8 * P], F32)       # rowsums, local-q linear
            ones_sb = persist.tile([P, 1], BF16)
            warm_w = persist.tile([P, P], BF16)
            warm_x = persist.tile([P, 256], BF16)

            # Three parallel DMA streams. The two tensors gating phase A (m2,
            # xtq) lead the two queues that flow earliest (sync HWDGE and
            # gpsimd SWDGE); the scalar HWDGE queue gets arbitration-starved
            # behind them, so it carries only tensors needed later (masks at
            # first S-block, wv at first O).
            nc.sync.dma_start(out=m_sb[:, 0:2, :, :], in_=m2[0, :, :])
            nc.sync.dma_start(out=xtq_sb[:, 0], in_=xtq[0, :, :])
            nc.sync.dma_start(out=m_sb[:, 2:4, :, :], in_=m2[1, :, :])
            nc.sync.dma_start(out=m_sb[:, 4:6, :, :], in_=m2[2, :, :])
            nc.sync.dma_start(out=m_sb[:, 6:8, :, :], in_=m2[3, :, :])
            nc.sync.dma_start(out=xtq_sb[:, 1], in_=xtq[1, :, :])
            nc.sync.dma_start(out=xtq_sb[:, 2], in_=xtq[2, :, :])
            nc.sync.dma_start(out=xtq_sb[:, 3], in_=xtq[3, :, :])
            nc.sync.dma_start(out=xt_sb[:, 0], in_=xt[0, :, :])
            nc.sync.dma_start(out=xt_sb[:, 1], in_=xt[1, :, :])
            nc.gpsimd.dma_start(out=u_sb, in_=ud[:, :])
            nc.gpsimd.dma_start(out=mask_sb, in_=maskd[:, :])


            nc.vector.memset(ones_sb, 1.0)
            nc.vector.memset(warm_w, 1.0)
            nc.vector.memset(warm_x, 0.125)

            with tc.tile_pool(name="bwork", bufs=2) as bwork, \
                 tc.tile_pool(name="ps", bufs=1, space="PSUM") as ps:

                # HAM warmup + DMA-window filler: keeps the PE busy (and at
                # 2.4 GHz) until the first real operands land.
                for i in range(NSPIN):
                    junk = ps.tile([P, 256], F32, tag="s", bufs=3, name="junk")
                    nc.tensor.matmul(junk, warm_w, warm_x, start=True, stop=True)

                # ---------------- Phase A: H = M^T x_q^T + u ----------------
                ztags = ["za", "zb", "oa", "ob"]
                for qq in range(4):
                    for dt in range(8):
                        pg = ps.tile([P, 256], F32, tag=ztags[(qq * 8 + dt) % 4],
                                     name="pg")
                        for ct in range(8):
                            nc.tensor.matmul(
                                pg,
                                m_sb[:, dt, ct, :],
                                xtq_sb[:, qq, ct, :],
                                start=(ct == 0),
                                stop=(ct == 7),
                            )
                        nc.vector.tensor_scalar_add(
                            h_sb[:, dt, qq * 256:(qq + 1) * 256], pg,
                            u_sb[:, dt:dt + 1])
                        if qq == 0 and dt == 6:
                            # xr/wv are not needed until ~40us, but their
                            # large packets win the DMA engines' packet
                            # round-robin and starve the phase-A gating
                            # transfers. Hold them until G's second quarter:
                            # the touches below READ the h chunk written
                            # above (a real RAW dep -- the tile scheduler
                            # reorders anything without one), and the DMAs
                            # wait on the touches via WAW overlap.
                            for q4 in range(4):
                                nc.vector.tensor_copy(
                                    xr_sb[:, 4 * q4, 0:1],
                                    h_sb[:, dt, qq * 256:qq * 256 + 1])
                            nc.vector.tensor_copy(
                                wv_sb[:, 0, 0:1],
                                h_sb[:, dt, qq * 256:qq * 256 + 1])
                            for q4 in range(4):
                                nc.scalar.dma_start(
                                    out=xr_sb[:, 4 * q4:4 * q4 + 4, :],
                                    in_=xr[q4, :, :])
                            nc.scalar.dma_start(out=wv_sb, in_=wvd[:, :])

                # ------------- Phase B: attention per q-tile pair -----------
                # PSUM start=True zeroes a whole 2KB bank region, so every
                # accumulating tile gets its own bank: Z^T runs as 4 passes of
                # 2 slices on tags za..ob (1 bank each); the O matmuls reuse
                # za/zb right after pass 3's copies; the next pair's pass 0
                # reuses them again after the o copies. All sequential.
                for p in range(4):
                    K = 4 * p + 4
                    rs_ps = ps.tile([1, 256], F32, tag="rs", name="rs_ps")

                    def s_block(kt):
                        s_ps = ps.tile([P, 256], F32, tag="s", bufs=3, name="s_ps")
                        for ct in range(8):
                            nc.tensor.matmul(
                                s_ps,
                                xt_sb[:, kt // 8, ct, (kt % 8) * P:(kt % 8 + 1) * P],
                                h_sb[:, ct, p * 256:(p + 1) * 256],
                                start=(ct == 0),
                                stop=(ct == 7),
                            )
                        if kt >= 4 * p:
                            nc.vector.tensor_add(s_ps, s_ps,
                                                 mask_sb[:, kt - 4 * p, :])
                        nc.scalar.activation(p_sb[:, kt, :], s_ps, AF.Exp,
                                             scale=SCALE)

                    def z_pass(j, zp):
                        # slices 2j, 2j+1 of Z^T; rowsum rides pass 0
                        for kt in range(K):
                            for sl in range(2):
                                s = 2 * j + sl
                                nc.tensor.matmul(
                                    zp[sl],
                                    xr_sb[:, kt, s * P:(s + 1) * P],
                                    p_sb[:, kt, :],
                                    start=(kt == 0),
                                    stop=(kt == K - 1),
                                )
                            if j == 0:
                                nc.tensor.matmul(rs_ps, ones_sb, p_sb[:, kt, :],
                                                 start=(kt == 0),
                                                 stop=(kt == K - 1))
                        for sl in range(2):
                            nc.vector.tensor_copy(zt_sb[:, 2 * j + sl, :], zp[sl])

                    # All S blocks first (exp chases one block behind on the
                    # scalar engine), then the Z passes stream P^T. Pass tags
                    # alternate za/zb <-> oa/ob so pass j+1 never waits on
                    # pass j's PSUM->SBUF copies.
                    for kt in range(K):
                        s_block(kt)
                    for j in range(4):
                        z_pass(j, [ps.tile([P, 256], F32,
                                           tag=ztags[2 * (j % 2) + sl],
                                           name="zp") for sl in range(2)])
                    nc.scalar.copy(rs_sb[:, p * 256:(p + 1) * 256], rs_ps)
                    if p == 3:
                        nc.sync.dma_start(out=rsd[:, :], in_=rs_sb)

                    # O for pair p: zt copies chase the Z passes, so the s=0..5
                    # reads are ready and 6..7 land under the first O matmuls.
                    for qh in range(2):
                        o_sb = bwork.tile([P, D], BF16, tag="o")
                        for dc in range(2):
                            po = ps.tile([P, 512], F32, tag=ztags[dc], name="po")
                            for s in range(8):
                                nc.tensor.matmul(
                                    po,
                                    zt_sb[:, s, qh * P:(qh + 1) * P],
                                    wv_sb[:, s, dc * 512:(dc + 1) * 512],
                                    start=(s == 0),
                                    stop=(s == 7),
                                )
                            nc.scalar.copy(o_sb[:, dc * 512:(dc + 1) * 512], po)
                            t = 2 * p + qh
                            nc.sync.dma_start(
                                out=outd[t * P:(t + 1) * P,
                                         dc * 512:(dc + 1) * 512],
                                in_=o_sb[:, dc * 512:(dc + 1) * 512])


    nc.finalize()
    return nc


def _prep_inputs(x, wq, bq, wk, bk, wv, bv):
    bf = ml_dtypes.bfloat16
    wq32 = np.asarray(wq, np.float32)
    wk32 = np.asarray(wk, np.float32)
    m_host = (wq32 @ wk32.T).astype(bf)                 # Wq Wk^T  [D, D]
    u_host = (wk32 @ np.asarray(bq, np.float32))        # Wk bq    [D]

    # M in dt-major chunks: m2[dh][p, dtm*1024 + ct*128 + c]
    #   = M[ct*128+p, (dh*4+dtm)*128 + c]
    m2 = (m_host.reshape(8, P, 8, P).transpose(2, 1, 0, 3)
          .reshape(4, 2, P, 8, P).transpose(0, 2, 1, 3, 4)
          .reshape(4, P, 2048))
    m2 = np.ascontiguousarray(m2)

    wv_b = (np.asarray(wv, np.float32).astype(bf)
            .reshape(8, P, 1024).transpose(1, 0, 2).reshape(P, 8192))
    wv_b = np.ascontiguousarray(wv_b)
    u2 = np.ascontiguousarray(u_host.reshape(8, P).T).astype(np.float32)

    # Diagonal-region masks per h: keep iff r*128+ki <= (h+2*jh)*128 + j
    ki = np.arange(P)[:, None, None]
    r = np.arange(4)[None, :, None]
    j = np.arange(256)[None, None, :]
    masks = [np.where(r * P + ki <= (h + 2 * (j // P)) * P + (j % P),
                      0.0, NEG).astype(np.float32).reshape(P, 4 * 256)
             for h in range(2)]

    in_maps = []
    for core in range(8):
        b, h = core // 2, core % 2
        xb = np.asarray(x[b], np.float32)
        xbT = xb.T.astype(bf)                            # [D, L]
        # x^T token-halves: xt[half][p, ct*1024+tok]
        xt_d = np.ascontiguousarray(
            xbT.reshape(8, P, 2, 1024).transpose(2, 1, 0, 3).reshape(2, P, 8192))
        # q-columns of x^T (local tile t -> global 2t+h), qc-halves
        qcols = ((np.arange(8)[:, None] * 2 + h) * P
                 + np.arange(P)[None, :]).ravel()
        xqT = np.ascontiguousarray(xbT[:, qcols])        # [D, 1024]
        xtq_d = np.ascontiguousarray(
            xqT.reshape(8, P, 4, 256).transpose(2, 1, 0, 3).reshape(4, P, 2048))
        # x rows in kt-quarters: xr[q4][p, kti*1024+d]
        xr_d = np.ascontiguousarray(
            xb.astype(bf).reshape(4, 4, P, 1024).transpose(0, 2, 1, 3)
            .reshape(4, P, 4096))
        in_maps.append({
            "xt": xt_d, "xtq": xtq_d, "m2": m2, "xr": xr_d, "wv": wv_b,
            "u": u2, "mask": masks[h],
        })
    return in_maps


def kernel(x, wq, bq, wk, bk, wv, bv, _trace=False, _trace_kwargs=None):
    if "nc" not in _CACHED:
        _CACHED["nc"] = build_nc()
    nc = _CACHED["nc"]
    in_maps = _prep_inputs(x, wq, bq, wk, bk, wv, bv)
    kw = {}
    if _trace:
        kw = dict(trace=True, **(_trace_kwargs or {}))
    res = run_bass_kernel_spmd(nc, in_maps, list(range(8)), **kw)
    bv32 = np.asarray(bv, np.float32)
    out = np.empty((B, L, D), np.float32)
    for core in range(8):
        b, h = core // 2, core % 2
        o = np.asarray(res.results[core]["out"]).astype(np.float32)
        rs = np.asarray(res.results[core]["rs"], np.float32).reshape(8 * P, 1)
        o = o / rs + bv32[None, :]
        out[b].reshape(16, P, D)[h::2] = o.reshape(8, P, D)
    if _trace:
        _CACHED["last_results"] = res
    return out


# revision 21
# speedup vs baseline: 1.0456x; 1.0239x over previous
"""MiniCausalAttention on 8 NeuronCores (Trainium2, Bass/Tile).

Problem: x[4,2048,1024] fp32; q/k/v = x@w+b; causal softmax(q k^T/sqrt(D)) @ v.

Sharding: 8 cores = (batch b in 0..3) x (half h in 0..1). Core (b,h) handles
query tiles g = 2t+h for local t in 0..7 (interleaved 128-row tiles), so every
core sees the same causal extents -> one SPMD program, perfectly balanced.

Transpose-free formulation. With M = Wq Wk^T and u = Wk bq (host-precomputed),
the softmax-effective scores are S = x_q M x^T + 1 (x) (x u)^T (per-query
terms cancel). Computing the TRANSPOSED scores instead,
  S^T = x (M^T x_q^T + u 1^T) = x H,      H = G^T + u 1^T,  G^T = M^T x_q^T,
makes every downstream operand come out in exactly the layout the next matmul
needs, so the kernel contains zero PE transposes:
  phase A   G^T[d,q]   = lhsT(m) @ x_q^T        -> +u -> H in SBUF (bf16)
  S-step    S^T[k,q]   = lhsT(x^T) @ H          -> mask+exp -> P^T (bf16)
  Z-step    Z^T[d',q]  = lhsT(x rows) @ P^T      (attention @ x, un-projected)
  O-step    O[q,d]     = lhsT(Z^T) @ Wv          (query-major, DMA-ready)
Row-sums ride the Z-step as a 1-column ones matmul; normalization (/rowsum,
+bv) runs on the host after gather (O(L*D) work, like the unshard reshape).

q-tiles are processed in pairs (256-col moving operands) over key tiles
kt < 4p+4 with per-core mask data resolving h and the diagonal. PSUM start=True
zeroes the whole 2KB bank, so every accumulating tile owns a bank (8 banks:
S x3, Z-pass tags x4, rowsum x1); Z^T runs as 4 passes of 2 slices, and the
O matmuls reuse two Z tags right after the pass-3 copies. Input DMAs are laid
out fully contiguous per partition (big packets win the DMA engines' packet
round-robin) with the not-yet-needed xr/wv transfers data-gated behind phase
A's second quarter so the phase-A gating transfers get the early bandwidth.
"""

import sys

if "/opt/trn_rl_repo" not in sys.path:
    sys.path.insert(0, "/opt/trn_rl_repo")

import numpy as np
import ml_dtypes

import concourse.bass as bass  # noqa: F401
import concourse.tile as tile
from concourse import bacc, mybir
from concourse.bass_utils import run_bass_kernel_spmd

BF16 = mybir.dt.bfloat16
F32 = mybir.dt.float32
AF = mybir.ActivationFunctionType

B, L, D = 4, 2048, 1024
P = 128
SCALE = 1.0 / 32.0   # 1/sqrt(D)
NEG = -30000.0       # exp(SCALE*NEG) == 0 in f32
NSPIN = 22           # PE warmup spins covering the input-DMA window

_CACHED = {}


def build_nc():
    nc = bacc.Bacc(None, target_bir_lowering=False)

    xt = nc.declare_dram_parameter("xt", [2, P, 8 * 1024], BF16, isOutput=False)
    xtq = nc.declare_dram_parameter("xtq", [4, P, 2 * 1024], BF16, isOutput=False)
    m2 = nc.declare_dram_parameter("m2", [4, P, 2 * 1024], BF16, isOutput=False)
    xr = nc.declare_dram_parameter("xr", [4, P, 4 * 1024], BF16, isOutput=False)
    wvd = nc.declare_dram_parameter("wv", [P, 8 * 1024], BF16, isOutput=False)
    ud = nc.declare_dram_parameter("u", [P, 8], F32, isOutput=False)
    maskd = nc.declare_dram_parameter("mask", [P, 4 * 256], F32, isOutput=False)
    outd = nc.declare_dram_parameter("out", [8 * P, D], BF16, isOutput=True)
    rsd = nc.declare_dram_parameter("rs", [1, 8 * P], F32, isOutput=True)

    with tile.TileContext(nc) as tc:
        with tc.tile_pool(name="persist", bufs=1) as persist:
            # DMA-contiguous layouts: each input DMA writes one fully
            # contiguous free-range per partition (8-16KB packets; strided
            # packets starve in the DMA engines' packet round-robin).
            xt_sb = persist.tile([P, 2, 8, 1024], BF16)  # x^T: [d, half, ct, tok]
            xtq_sb = persist.tile([P, 4, 8, 256], BF16)  # x_q^T: [d, qq, ct, q]
            m_sb = persist.tile([P, 8, 8, P], BF16)     # M: [d'-in-chunk, dt, ct, dcol]
            xr_sb = persist.tile([P, 16, 1024], BF16)   # x rows: [tok-in-tile, kt, d]
            wv_sb = persist.tile([P, 8, 1024], BF16)    # Wv: [d'-in-chunk, s, d]
            u_sb = persist.tile([P, 8], F32)            # Wk bq: [d-in-chunk, dt]
            mask_sb = persist.tile([P, 4, 256], F32)    # diag masks: [k, r, qcol]
            h_sb = persist.tile([P, 8, 1024], BF16)     # H = G^T + u: [d, dt, qcol]
            p_sb = persist.tile([P, 16, 256], BF16)     # P^T per pair: [k, kt, qcol]
            zt_sb = persist.tile([P, 8, 256], BF16)     # Z^T: [d'-in-chunk, s, qcol]
            rs_sb = persist.tile([1, 8 * P], F32)       # rowsums, local-q linear
            ones_sb = persist.tile([P, 1], BF16)
            warm_w = persist.tile([P, P], BF16)
            warm_x = persist.tile([P, 256], BF16)

            # Three parallel DMA streams. The two tensors gating phase A (m2,
            # xtq) lead the two queues that flow earliest (sync HWDGE and
            # gpsimd SWDGE); the scalar HWDGE queue gets arbitration-starved
            # behind them, so it carries only tensors needed later (masks at
            # first S-block, wv at first O).
            nc.sync.dma_start(out=m_sb[:, 0:2, :, :], in_=m2[0, :, :])
            nc.sync.dma_start(out=xtq_sb[:, 0], in_=xtq[0, :, :])
            nc.sync.dma_start(out=m_sb[:, 2:4, :, :], in_=m2[1, :, :])
            nc.sync.dma_start(out=m_sb[:, 4:6, :, :], in_=m2[2, :, :])
            nc.sync.dma_start(out=m_sb[:, 6:8, :, :], in_=m2[3, :, :])
            nc.sync.dma_start(out=xtq_sb[:, 1], in_=xtq[1, :, :])
            nc.sync.dma_start(out=xtq_sb[:, 2], in_=xtq[2, :, :])
            nc.sync.dma_start(out=xtq_sb[:, 3], in_=xtq[3, :, :])
            nc.sync.dma_start(out=xt_sb[:, 0], in_=xt[0, :, :])
            nc.sync.dma_start(out=xt_sb[:, 1], in_=xt[1, :, :])
            nc.gpsimd.dma_start(out=u_sb, in_=ud[:, :])


            nc.vector.memset(ones_sb, 1.0)
            nc.vector.memset(warm_w, 1.0)
            nc.vector.memset(warm_x, 0.125)

            with tc.tile_pool(name="bwork", bufs=2) as bwork, \
                 tc.tile_pool(name="ps", bufs=1, space="PSUM") as ps:

                # HAM warmup + DMA-window filler: keeps the PE busy (and at
                # 2.4 GHz) until the first real operands land.
                for i in range(NSPIN):
                    junk = ps.tile([P, 256], F32, tag="s", bufs=3, name="junk")
                    nc.tensor.matmul(junk, warm_w, warm_x, start=True, stop=True)

                # ---------------- Phase A: H = M^T x_q^T + u ----------------
                ztags = ["za", "zb", "oa", "ob"]
                for qq in range(4):
                    for dt in range(8):
                        pg = ps.tile([P, 256], F32, tag=ztags[(qq * 8 + dt) % 4],
                                     name="pg")
                        for ct in range(8):
                            nc.tensor.matmul(
                                pg,
                                m_sb[:, dt, ct, :],
                                xtq_sb[:, qq, ct, :],
                                start=(ct == 0),
                                stop=(ct == 7),
                            )
                        nc.vector.tensor_scalar_add(
                            h_sb[:, dt, qq * 256:(qq + 1) * 256], pg,
                            u_sb[:, dt:dt + 1])
                        if qq == 0 and dt == 6:
                            # xr/wv are not needed until ~40us, but their
                            # large packets win the DMA engines' packet
                            # round-robin and starve the phase-A gating
                            # transfers. Hold them until G's second quarter:
                            # the touches below READ the h chunk written
                            # above (a real RAW dep -- the tile scheduler
                            # reorders anything without one), and the DMAs
                            # wait on the touches via WAW overlap.
                            for q4 in range(4):
                                nc.vector.tensor_copy(
                                    xr_sb[:, 4 * q4, 0:1],
                                    h_sb[:, dt, qq * 256:qq * 256 + 1])
                            nc.vector.tensor_copy(
                                wv_sb[:, 0, 0:1],
                                h_sb[:, dt, qq * 256:qq * 256 + 1])
                            nc.vector.tensor_copy(
                                mask_sb[:, 0, 0:1],
                                h_sb[:, dt, qq * 256:qq * 256 + 1])
                            nc.scalar.dma_start(out=mask_sb,
                                                in_=maskd[:, :])
                            nc.scalar.dma_start(
                                out=xr_sb[:, 0:4, :], in_=xr[0, :, :])
                            nc.scalar.dma_start(out=wv_sb, in_=wvd[:, :])
                            for q4 in range(1, 4):
                                nc.scalar.dma_start(
                                    out=xr_sb[:, 4 * q4:4 * q4 + 4, :],
                                    in_=xr[q4, :, :])

                # ------------- Phase B: attention per q-tile pair -----------
                # PSUM start=True zeroes a whole 2KB bank region, so every
                # accumulating tile gets its own bank: Z^T runs as 4 passes of
                # 2 slices on tags za..ob (1 bank each); the O matmuls reuse
                # za/zb right after pass 3's copies; the next pair's pass 0
                # reuses them again after the o copies. All sequential.
                for p in range(4):
                    K = 4 * p + 4
                    rs_ps = ps.tile([1, 256], F32, tag="rs", name="rs_ps")

                    def s_block(kt):
                        s_ps = ps.tile([P, 256], F32, tag="s", bufs=3, name="s_ps")
                        for ct in range(8):
                            nc.tensor.matmul(
                                s_ps,
                                xt_sb[:, kt // 8, ct, (kt % 8) * P:(kt % 8 + 1) * P],
                                h_sb[:, ct, p * 256:(p + 1) * 256],
                                start=(ct == 0),
                                stop=(ct == 7),
                            )
                        if kt >= 4 * p:
                            nc.vector.tensor_add(s_ps, s_ps,
                                                 mask_sb[:, kt - 4 * p, :])
                        nc.scalar.activation(p_sb[:, kt, :], s_ps, AF.Exp,
                                             scale=SCALE)

                    def z_pass(j, zp):
                        # slices 2j, 2j+1 of Z^T; rowsum rides pass 0
                        for kt in range(K):
                            for sl in range(2):
                                s = 2 * j + sl
                                nc.tensor.matmul(
                                    zp[sl],
                                    xr_sb[:, kt, s * P:(s + 1) * P],
                                    p_sb[:, kt, :],
                                    start=(kt == 0),
                                    stop=(kt == K - 1),
                                )
                            if j == 0:
                                nc.tensor.matmul(rs_ps, ones_sb, p_sb[:, kt, :],
                                                 start=(kt == 0),
                                                 stop=(kt == K - 1))
                        for sl in range(2):
                            nc.vector.tensor_copy(zt_sb[:, 2 * j + sl, :], zp[sl])

                    # All S blocks first (exp chases one block behind on the
                    # scalar engine), then the Z passes stream P^T. Pass tags
                    # alternate za/zb <-> oa/ob so pass j+1 never waits on
                    # pass j's PSUM->SBUF copies.
                    for kt in range(K):
                        s_block(kt)
                    for j in range(4):
                        z_pass(j, [ps.tile([P, 256], F32,
                                           tag=ztags[2 * (j % 2) + sl],
                                           name="zp") for sl in range(2)])
                    nc.scalar.copy(rs_sb[:, p * 256:(p + 1) * 256], rs_ps)
                    if p == 3:
                        nc.sync.dma_start(out=rsd[:, :], in_=rs_sb)

                    # O for pair p: zt copies chase the Z passes, so the s=0..5
                    # reads are ready and 6..7 land under the first O matmuls.
                    for qh in range(2):
                        o_sb = bwork.tile([P, D], BF16, tag="o")
                        for dc in range(2):
                            po = ps.tile([P, 512], F32, tag=ztags[dc], name="po")
                            for s in range(8):
                                nc.tensor.matmul(
                                    po,
                                    zt_sb[:, s, qh * P:(qh + 1) * P],
                                    wv_sb[:, s, dc * 512:(dc + 1) * 512],
                                    start=(s == 0),
                                    stop=(s == 7),
                                )
                            nc.scalar.copy(o_sb[:, dc * 512:(dc + 1) * 512], po)
                            t = 2 * p + qh
                            nc.sync.dma_start(
                                out=outd[t * P:(t + 1) * P,
                                         dc * 512:(dc + 1) * 512],
                                in_=o_sb[:, dc * 512:(dc + 1) * 512])


    nc.finalize()
    return nc


def _prep_inputs(x, wq, bq, wk, bk, wv, bv):
    bf = ml_dtypes.bfloat16
    wq32 = np.asarray(wq, np.float32)
    wk32 = np.asarray(wk, np.float32)
    m_host = (wq32 @ wk32.T).astype(bf)                 # Wq Wk^T  [D, D]
    u_host = (wk32 @ np.asarray(bq, np.float32))        # Wk bq    [D]

    # M in dt-major chunks: m2[dh][p, dtm*1024 + ct*128 + c]
    #   = M[ct*128+p, (dh*4+dtm)*128 + c]
    m2 = (m_host.reshape(8, P, 8, P).transpose(2, 1, 0, 3)
          .reshape(4, 2, P, 8, P).transpose(0, 2, 1, 3, 4)
          .reshape(4, P, 2048))
    m2 = np.ascontiguousarray(m2)

    wv_b = (np.asarray(wv, np.float32).astype(bf)
            .reshape(8, P, 1024).transpose(1, 0, 2).reshape(P, 8192))
    wv_b = np.ascontiguousarray(wv_b)
    u2 = np.ascontiguousarray(u_host.reshape(8, P).T).astype(np.float32)

    # Diagonal-region masks per h: keep iff r*128+ki <= (h+2*jh)*128 + j
    ki = np.arange(P)[:, None, None]
    r = np.arange(4)[None, :, None]
    j = np.arange(256)[None, None, :]
    masks = [np.where(r * P + ki <= (h + 2 * (j // P)) * P + (j % P),
                      0.0, NEG).astype(np.float32).reshape(P, 4 * 256)
             for h in range(2)]

    in_maps = []
    for core in range(8):
        b, h = core // 2, core % 2
        xb = np.asarray(x[b], np.float32)
        xbT = xb.T.astype(bf)                            # [D, L]
        # x^T token-halves: xt[half][p, ct*1024+tok]
        xt_d = np.ascontiguousarray(
            xbT.reshape(8, P, 2, 1024).transpose(2, 1, 0, 3).reshape(2, P, 8192))
        # q-columns of x^T (local tile t -> global 2t+h), qc-halves
        qcols = ((np.arange(8)[:, None] * 2 + h) * P
                 + np.arange(P)[None, :]).ravel()
        xqT = np.ascontiguousarray(xbT[:, qcols])        # [D, 1024]
        xtq_d = np.ascontiguousarray(
            xqT.reshape(8, P, 4, 256).transpose(2, 1, 0, 3).reshape(4, P, 2048))
        # x rows in kt-quarters: xr[q4][p, kti*1024+d]
        xr_d = np.ascontiguousarray(
            xb.astype(bf).reshape(4, 4, P, 1024).transpose(0, 2, 1, 3)
            .reshape(4, P, 4096))
        in_maps.append({
            "xt": xt_d, "xtq": xtq_d, "m2": m2, "xr": xr_d, "wv": wv_b,
            "u": u2, "mask": masks[h],
        })
    return in_maps


def kernel(x, wq, bq, wk, bk, wv, bv, _trace=False, _trace_kwargs=None):
    if "nc" not in _CACHED:
        _CACHED["nc"] = build_nc()
    nc = _CACHED["nc"]
    in_maps = _prep_inputs(x, wq, bq, wk, bk, wv, bv)
    kw = {}
    if _trace:
        kw = dict(trace=True, **(_trace_kwargs or {}))
    res = run_bass_kernel_spmd(nc, in_maps, list(range(8)), **kw)
    bv32 = np.asarray(bv, np.float32)
    out = np.empty((B, L, D), np.float32)
    for core in range(8):
        b, h = core // 2, core % 2
        o = np.asarray(res.results[core]["out"]).astype(np.float32)
        rs = np.asarray(res.results[core]["rs"], np.float32).reshape(8 * P, 1)
        o = o / rs + bv32[None, :]
        out[b].reshape(16, P, D)[h::2] = o.reshape(8, P, D)
    if _trace:
        _CACHED["last_results"] = res
    return out
